# revision 23
# baseline (speedup 1.0000x reference)
"""Causal multi-head attention on 8 trn2 NeuronCores.

Problem: B=4, S=2048, D=1024, H=16 heads (HD=64), causal softmax attention
with out-projection + bias.

Sharding (tensor-parallel over heads, data-parallel over batch):
  core c -> batch b = c // 2, head half = c % 2 (8 of 16 heads, 512 dims).
  Every core runs the IDENTICAL program on different data:
    - xt   : x[b].T                  [1024, 2048] (host pre-transposed)
    - wq/wk/wv : W[:, half slice]    [1024, 512]
    - wot  : Wo[:, half slice].T     [512, 1024]
    - bo   : bias on even cores, zeros on odd cores  [1, 1024]
  Core output: partial out-projection [2048, 1024]; host sums the two
  partials per batch at unshard time (row-parallel out_proj reduction).

Kernel (per core), flash-style with transposed scores, globally
software-pipelined:

  The attention phase is ACT-bound (exp of ~18M causal scores) while the
  projection / out-projection phases are PE-bound.  Instead of running the
  phases back to back (PE idles behind ACT in attention; ACT idles in the
  projections), the emission interleaves them: a small prefix computes
  K/Q/V only for the first q-chunk, attention starts immediately, and the
  remaining projection chunks + the out-projection of finished q-chunks
  are spread through the attention units as PE filler that executes while
  the PE would otherwise wait on exp.

  Dtypes: projections run f32r x f32r (DRAM tensors declared f32r so plain
  HWDGE DMAs feed the PE -- no SWDGE cast DMAs).  All attention operands
  (Q/K/V/ex/ct) are bf16, produced for free by the PSUM->SBUF copies; PE
  rate is 1 row/cycle for both f32r and bf16, but bf16 halves SBUF.
  Causal masking: exp runs on the trimmed diagonal band, then a Pool
  (gpsimd) affine_select zeroes the sub-diagonal leftovers in-place in the
  bf16 exp tile (exp(-inf) never computed; masked weights are exact 0 so
  the ones-column denominator stays correct).  The out-proj bias is a
  [128,1024] broadcast tile added by DVE during the PSUM->SBUF copy
  (no rank-1 PE matmul).
"""

import os
from collections import deque
from contextlib import ExitStack

import numpy as np

import concourse.bass as bass
import concourse.mybir as mybir
import concourse.tile as tile
from concourse import bacc
from concourse.bass_utils import run_bass_kernel_spmd

B, S, D, H = 4, 2048, 1024, 16
HD = 64          # head dim
DL = 512         # local head dims per core (8 heads)
HH = 8           # local heads
P = 128
O_ = 1024        # output dims (full)
QC = 512         # q chunk (moving free dim)
N_QC = S // QC   # 4
N_DI = D // P    # 8
N_DL = DL // P   # 4
N_ST = S // P    # 16 seq tiles
VW = HD + 1      # 65: V columns + ones column
N_CORES = 8

F32 = mybir.dt.float32
F32R = mybir.dt.float32r
BF16 = mybir.dt.bfloat16


def build_nc():
    nc = bacc.Bacc("TRN2", target_bir_lowering=False, debug=False,
                   num_devices=N_CORES, num_swdge_queues=4)
    xt = nc.dram_tensor("xt", [D, S], F32, kind="ExternalInput").ap()
    wq = nc.dram_tensor("wq", [D, DL], F32, kind="ExternalInput").ap()
    wk = nc.dram_tensor("wk", [D, DL], F32, kind="ExternalInput").ap()
    wv = nc.dram_tensor("wv", [D, DL], F32, kind="ExternalInput").ap()
    wot = nc.dram_tensor("wot", [DL, O_], F32, kind="ExternalInput").ap()
    bo = nc.dram_tensor("bo", [1, O_], F32, kind="ExternalInput").ap()
    out = nc.dram_tensor("out", [S, O_], F32, kind="ExternalOutput").ap()

    repeat = int(os.environ.get("MHA_REPEAT", "1"))
    hwloop = int(os.environ.get("MHA_HWLOOP", "0"))
    with tile.TileContext(nc) as tc:
        if hwloop > 1:
            with tc.For_i(0, hwloop, 1):
                _emit(nc, tc, xt, wq, wk, wv, wot, bo, out)
        else:
            for _ in range(repeat):
                _emit(nc, tc, xt, wq, wk, wv, wot, bo, out)
    nc.compile()
    return nc


def _emit(nc, tc, xt, wq, wk, wv, wot, bo, out):
    Exp = mybir.ActivationFunctionType.Exp
    add = mybir.AluOpType.add
    mult = mybir.AluOpType.mult

    with ExitStack() as ctx:
        # ---- constants ------------------------------------------------------
        consts = ctx.enter_context(tc.tile_pool(name="consts", bufs=1))
        ones_f = consts.tile([P, P], F32, tag="ones_f")
        ones_v = ones_f[:, 0:HH]
        nc.gpsimd.memset(ones_f[:], 1.0)
        bo_sb = consts.tile([1, O_], F32, tag="bo_sb")
        nc.sync.dma_start(bo_sb[:], bo[:])
        bias_bc = consts.tile([P, O_], F32, tag="bias_bc")
        nc.gpsimd.partition_broadcast(bias_bc[:], bo_sb[:])
        EXFREE = bool(os.environ.get("MHA_EXFREE"))
        if EXFREE:
            konst = consts.tile([P, 2 * QC], F32, tag="konst")
            nc.gpsimd.memset(konst[:], 0.5)

        # ---- persistent storage --------------------------------------------
        xa_pool = ctx.enter_context(tc.tile_pool(name="xa", bufs=1))
        xb_pool = ctx.enter_context(tc.tile_pool(name="xb", bufs=1))
        w_pool = ctx.enter_context(tc.tile_pool(name="w", bufs=1))
        qt_pool = ctx.enter_context(tc.tile_pool(name="qt", bufs=1))
        kt_pool = ctx.enter_context(tc.tile_pool(name="kt", bufs=1))
        v_pool = ctx.enter_context(tc.tile_pool(name="v", bufs=1))
        ct_pool = ctx.enter_context(tc.tile_pool(name="ct", bufs=1))

        # x[di].T split at column 512: A = first q/k chunk, B = rest.  The
        # split lets the prefix projections start after only 2MB of x has
        # landed instead of 8MB.
        xa = [xa_pool.tile([P, QC], BF16, name=f"xa{i}", tag=f"xa{i}") for i in range(N_DI)]
        xb = [xb_pool.tile([P, S - QC], BF16, name=f"xb{i}", tag=f"xb{i}") for i in range(N_DI)]
        wk_sb = [w_pool.tile([P, DL], BF16, name=f"wk{i}", tag=f"wk{i}") for i in range(N_DI)]
        wq_sb = [w_pool.tile([P, DL], BF16, name=f"wq{i}", tag=f"wq{i}") for i in range(N_DI)]
        wv_sb = [w_pool.tile([P, DL], BF16, name=f"wv{i}", tag=f"wv{i}") for i in range(N_DI)]
        wot_sb = [w_pool.tile([P, O_], BF16, name=f"wot{j}", tag=f"wot{j}") for j in range(N_DL)]
        qt_t = [qt_pool.tile([P, S], BF16, name=f"qt{j}", tag=f"qt{j}") for j in range(N_DL)]
        kt_t = [kt_pool.tile([P, S], BF16, name=f"kt{j}", tag=f"kt{j}") for j in range(N_DL)]
        v_t = [v_pool.tile([P, HH * VW], BF16, name=f"v{i}", tag=f"v{i}") for i in range(N_ST)]
        ct_t = [ct_pool.tile([P, S], BF16, name=f"ct{j}", tag=f"ct{j}") for j in range(N_DL)]

        # DMA issue order == completion order: the prefix needs xa+wk+wq+wv.
        for i in range(N_DI):
            nc.gpsimd.dma_start(xa[i][:], xt[i * P:(i + 1) * P, 0:QC])
        for i in range(N_DI):
            nc.gpsimd.dma_start(wk_sb[i][:], wk[i * P:(i + 1) * P, :])
        for i in range(N_DI):
            nc.gpsimd.dma_start(wq_sb[i][:], wq[i * P:(i + 1) * P, :])
        for i in range(N_DI):
            nc.gpsimd.dma_start(wv_sb[i][:], wv[i * P:(i + 1) * P, :])
        for i in range(N_DI):
            nc.gpsimd.dma_start(xb[i][:], xt[i * P:(i + 1) * P, QC:S])
        for j in range(N_DL):
            # gpsimd (SWDGE) DMA casts f32 -> bf16 in flight
            nc.gpsimd.dma_start(wot_sb[j][:], wot[j * P:(j + 1) * P, :])

        def xcols(c0, c1):
            """list of (xtile, lo, hi) covering x columns [c0, c1)."""
            if c1 <= QC:
                return [(xa, c0, c1)]
            if c0 >= QC:
                return [(xb, c0 - QC, c1 - QC)]
            return [(xa, c0, QC), (xb, 0, c1 - QC)]

        # ---- PSUM + working pools ------------------------------------------
        pps = ctx.enter_context(tc.tile_pool(name="pps", bufs=2, space="PSUM"))
        scps = ctx.enter_context(tc.tile_pool(name="scps", bufs=2, space="PSUM"))
        ctxps = ctx.enter_context(tc.tile_pool(name="ctxps", bufs=2, space="PSUM"))
        exp_pool = ctx.enter_context(
            tc.tile_pool(name="exp", bufs=int(os.environ.get("MHA_EXBUFS", "8"))))
        z_pool = ctx.enter_context(tc.tile_pool(name="zp", bufs=2))
        out_pool = ctx.enter_context(tc.tile_pool(name="outp", bufs=3))

        # ---- PE work quanta -------------------------------------------------
        def k_chunk(kc, dq):
            ps = pps.tile([P, QC], F32, name="pp", tag="pp")
            for di in range(N_DI):
                for xg, lo, hi in xcols(kc * QC, (kc + 1) * QC):
                    nc.tensor.matmul(
                        ps[:], wk_sb[di][:, dq * P:(dq + 1) * P], xg[di][:, lo:hi],
                        start=(di == 0), stop=(di == N_DI - 1))
            nc.vector.tensor_copy(kt_t[dq][:, kc * QC:(kc + 1) * QC], ps[:])

        def q_chunk(qc, dq):
            ps = pps.tile([P, QC], F32, name="pp", tag="pp")
            for di in range(N_DI):
                for xg, lo, hi in xcols(qc * QC, (qc + 1) * QC):
                    nc.tensor.matmul(
                        ps[:], wq_sb[di][:, dq * P:(dq + 1) * P], xg[di][:, lo:hi],
                        start=(di == 0), stop=(di == N_DI - 1))
            nc.vector.tensor_copy(qt_t[dq][:, qc * QC:(qc + 1) * QC], ps[:])

        def v_chunk(st):
            ps = pps.tile([P, DL], F32, name="pp", tag="pp")
            for di in range(N_DI):
                (xg, lo, hi), = xcols(st * P, (st + 1) * P)
                nc.tensor.matmul(
                    ps[:], xg[di][:, lo:hi], wv_sb[di][:],
                    start=(di == 0), stop=(di == N_DI - 1))
            vv = v_t[st].rearrange("p (h w) -> p h w", w=VW)
            nc.vector.tensor_copy(vv[:, :, 0:HD],
                                  ps.rearrange("p (h w) -> p h w", w=HD))
            nc.vector.tensor_copy(vv[:, :, HD:VW],
                                  ones_v.rearrange("p (h o) -> p h o", o=1))

        def out_chunk(qt):
            ob = out_pool.tile([P, O_], F32, name="ob", tag="ob")
            for oc in range(2):
                ps = pps.tile([P, QC], F32, name="pp", tag="pp")
                for dl in range(N_DL):
                    nc.tensor.matmul(
                        ps[:], ct_t[dl][:, qt * P:(qt + 1) * P],
                        wot_sb[dl][:, oc * QC:(oc + 1) * QC],
                        start=(dl == 0), stop=(dl == N_DL - 1))
                nc.vector.tensor_tensor(
                    ob[:, oc * QC:(oc + 1) * QC], ps[:],
                    bias_bc[:, oc * QC:(oc + 1) * QC], add)
            nc.sync.dma_start(out[qt * P:(qt + 1) * P, :], ob[:])

        # ---- attention units ------------------------------------------------
        state = {}
        norm_done = [0] * N_QC   # heads fully normalized per qc
        # timing-probe toggles (break correctness; for differential benching)
        NOEXP = bool(os.environ.get("MHA_NOEXP"))
        NOMASK = bool(os.environ.get("MHA_NOMASK"))
        NONORM = bool(os.environ.get("MHA_NONORM"))

        def emit_scores(u):
            qc, h, g, ng = u
            hr = slice(HD * (h % 2), HD * (h % 2) + HD)
            ht = h // 2
            sc = scps.tile([P, 2 * QC], F32, name="sc", tag="sc")
            ex = exp_pool.tile([P, 2 * QC], BF16, name="ex", tag="ex")
            offs = []
            for j in (0, 1):
                kt = 2 * g + j
                d = max(0, kt * P - qc * QC)   # masked q prefix width
                offs.append(d)
                nc.tensor.matmul(
                    sc[:, j * QC + d:(j + 1) * QC],
                    kt_t[ht][hr, kt * P:(kt + 1) * P],
                    qt_t[ht][hr, qc * QC + d:(qc + 1) * QC],
                    start=True, stop=True)
            src = konst if EXFREE else sc
            if NOEXP:
                pass
            elif offs[0] == offs[1] == 0:
                nc.scalar.activation(ex[:], src[:], Exp, scale=0.125)
            else:
                for j in (0, 1):
                    d = offs[j]
                    nc.scalar.activation(
                        ex[:, j * QC + d:(j + 1) * QC],
                        src[:, j * QC + d:(j + 1) * QC], Exp, scale=0.125)
            # zero the sub-diagonal leftovers of diagonal-band tiles in
            # place (Pool): keep where qlocal - k - 128*band >= 0, i.e.
            # base = d - 128*band, channel_multiplier = -1, col coeff +1.
            # The wedge only spans cols [d, d+128) of the slice (for
            # qlocal >= 128*band + 127 every k is kept), so select just
            # that 128-wide window.
            for j in (0, 1):
                if NOMASK:
                    break
                kt = 2 * g + j
                band = kt - 4 * qc
                if band < 0:
                    continue
                d = offs[j]
                nc.gpsimd.affine_select(
                    out=ex[:, j * QC + d:j * QC + d + P],
                    in_=ex[:, j * QC + d:j * QC + d + P],
                    pattern=[[1, P]],
                    compare_op=mybir.AluOpType.is_ge,
                    fill=0.0, base=d - P * band, channel_multiplier=-1)
            state[(qc, h, g)] = (sc, ex, offs)

        def emit_ctx(u):
            qc, h, g, ng = u
            ht = h // 2
            if g == 0:
                state[(qc, h, "ctx")] = ctxps.tile([P, QC], F32, name="ctx", tag="ctx")
            ctx_ps = state[(qc, h, "ctx")]
            sc, ex, offs = state.pop((qc, h, g))
            nkt = 2 * ng
            for j in (0, 1):
                kt = 2 * g + j
                d = offs[j]
                nc.tensor.matmul(
                    ctx_ps[0:VW, d:QC],
                    v_t[kt][:, h * VW:(h + 1) * VW],
                    ex[:, j * QC + d:(j + 1) * QC],
                    start=(kt == 0), stop=(kt == nkt - 1))
            if g == ng - 1:
                ctx_ps = state.pop((qc, h, "ctx"))
                hr = slice(HD * (h % 2), HD * (h % 2) + HD)
                if NONORM:
                    nc.vector.tensor_copy(
                        ct_t[ht][hr, qc * QC:(qc + 1) * QC], ctx_ps[0:HD, :])
                else:
                    rec = z_pool.tile([1, QC], F32, name="rec", tag="rec")
                    nc.vector.reciprocal(rec[:], ctx_ps[HD:VW, :])
                    rzb = z_pool.tile([HD, QC], F32, name="rzb", tag="rzb")
                    nc.gpsimd.partition_broadcast(rzb[:], rec[:])
                    nc.vector.tensor_tensor(
                        ct_t[ht][hr, qc * QC:(qc + 1) * QC],
                        ctx_ps[0:HD, :], rzb[:], mult)
                norm_done[qc] += 1

        # ---- schedule -------------------------------------------------------
        STAGGER = int(os.environ.get("MHA_STAGGER", "6"))

        # prefix: just enough projection for qc=0 attention to start
        for dq in range(N_DL):
            k_chunk(0, dq)
        for dq in range(N_DL):
            q_chunk(0, dq)
        for st in range(4):
            v_chunk(st)

        def fillers_for(qc):
            # qc0-qc2 are PE-bound (projection chunks must flow to feed the
            # next q-chunk), qc3 is ACT-bound (exp of the longest rows) --
            # park ALL out-projection quanta in qc3 where the PE has slack.
            fs = []
            if qc + 1 < N_QC:
                fs += [("k", qc + 1, dq) for dq in range(N_DL)]
                fs += [("q", qc + 1, dq) for dq in range(N_DL)]
                fs += [("v", st) for st in range(4 * (qc + 1), 4 * (qc + 2))]
            else:
                fs += [("o", qt) for qt in range(0, 4 * N_QC - 4)]
            return fs

        def run_filler(f):
            if f[0] == "k":
                k_chunk(f[1], f[2])
            elif f[0] == "q":
                q_chunk(f[1], f[2])
            elif f[0] == "v":
                v_chunk(f[1])
            else:
                out_chunk(f[1])

        pending = deque()
        for qc in range(N_QC):
            ng = 2 * (qc + 1)
            units = [(qc, h, g, ng) for h in range(HH) for g in range(ng)]
            fill = fillers_for(qc)
            n_u, n_f = len(units), len(fill)
            done_f = 0
            for i, u in enumerate(units):
                emit_scores(u)
                pending.append(u)
                if len(pending) > STAGGER:
                    emit_ctx(pending.popleft())
                while done_f < ((i + 1) * n_f) // n_u and fill:
                    f = fill[0]
                    if f[0] == "o" and norm_done[f[1] // 4] < HH:
                        # out-proj quantum not ready: rotate, try later
                        fill.append(fill.pop(0))
                        if all(g[0] == "o" for g in fill):
                            break
                        continue
                    run_filler(fill.pop(0))
                    done_f += 1
            # drain any fillers not emitted inside the unit loop
            for f in fill:
                if f[0] == "o":
                    # emitting an out-proj before its ct writes are emitted
                    # would deadlock the PE stream on hardware
                    assert norm_done[f[1] // 4] == HH, (qc, f)
                run_filler(f)

        while pending:
            emit_ctx(pending.popleft())
        for qt in range(4 * (N_QC - 1), 4 * N_QC):
            out_chunk(qt)


_NC_CACHE = None


def _get_nc():
    global _NC_CACHE
    if _NC_CACHE is None:
        _NC_CACHE = build_nc()
    return _NC_CACHE


def make_in_maps(x, Wq, Wk, Wv, Wo, bo):
    in_maps = []
    xts = [np.ascontiguousarray(x[b].T) for b in range(B)]
    zeros_bo = np.zeros((1, O_), np.float32)
    for c in range(N_CORES):
        b, half = c // 2, c % 2
        d0 = half * DL
        in_maps.append({
            "xt": xts[b],
            "wq": np.ascontiguousarray(Wq[:, d0:d0 + DL]),
            "wk": np.ascontiguousarray(Wk[:, d0:d0 + DL]),
            "wv": np.ascontiguousarray(Wv[:, d0:d0 + DL]),
            "wot": np.ascontiguousarray(Wo[:, d0:d0 + DL].T),
            "bo": bo.reshape(1, O_).astype(np.float32) if half == 0 else zeros_bo,
        })
    return in_maps


def kernel(x, Wq, Wk, Wv, Wo, bo):
    x = np.asarray(x, np.float32)
    Wq = np.asarray(Wq, np.float32)
    Wk = np.asarray(Wk, np.float32)
    Wv = np.asarray(Wv, np.float32)
    Wo = np.asarray(Wo, np.float32)
    bo = np.asarray(bo, np.float32)
    nc = _get_nc()
    in_maps = make_in_maps(x, Wq, Wk, Wv, Wo, bo)
    res = run_bass_kernel_spmd(nc, in_maps, core_ids=list(range(N_CORES)))
    out = np.empty((B, S, O_), np.float32)
    for b in range(B):
        out[b] = res.results[2 * b]["out"] + res.results[2 * b + 1]["out"]
    return out
